# revision 1
# baseline (speedup 1.0000x reference)
"""AggGraphCapsuleLayer kernel for 8 Trainium2 NeuronCores.

Strategy (per sharding hint): data-parallel over B' = batch*N/NN output
nodes. x (4, 32768, 8, 16) flattens to 131072 rows of [8, 16]; groups of
NN=8 consecutive rows form one output node -> 16384 nodes, 2048 per core.
W (8, 16, 256) is tiny and replicated. Routing is independent per node,
so there is no cross-device communication.

Per-shard math is the exact reference computation, jit-compiled for the
NeuronCore. The heavy pieces (projection x@W, routing einsums) lower to
PE matmuls/XLA fusions on-device.
"""

import jax
import jax.numpy as jnp
import numpy as np
from functools import partial

NUM_NEIGHBOURS = 8
NUM_CAPSULE = 16
DIM_CAPSULE = 16
NUM_ROUTING = 3
EPS = 1e-7

BATCH = 4
N_FULL = 32768
IC = 8
ID = 16
N_CORES = 8

# full node count and per-core shard
BP = BATCH * N_FULL // NUM_NEIGHBOURS      # 16384
BP_SHARD = BP // N_CORES                   # 2048
ROWS_SHARD = BP_SHARD * NUM_NEIGHBOURS     # 16384 rows of [IC, ID]


def _squash(v, axis=-1):
    s2 = jnp.sum(jnp.square(v), axis=axis, keepdims=True) + EPS
    scale = s2 / ((1.0 + s2) * jnp.sqrt(s2))
    return scale * v


def _shard_compute(xs, W):
    """xs: [ROWS_SHARD, IC, ID] fp32; W: [IC, ID, C*D]. -> [BP_SHARD, C, D]"""
    C, D, NN = NUM_CAPSULE, DIM_CAPSULE, NUM_NEIGHBOURS
    Bp = xs.shape[0] // NN
    R = NN * IC

    u = jnp.einsum('mip,ipq->miq', xs, W)
    u = u.reshape(Bp, R, C, D).transpose(0, 2, 1, 3)  # [Bp, C, R, D]

    b = jnp.zeros((Bp, C, R), dtype=u.dtype)
    outputs = None
    for i in range(NUM_ROUTING):
        c = jax.nn.softmax(b, axis=1)
        if i == NUM_ROUTING - 1:
            outputs = jnp.einsum('bcr,bcrd->bcd', c, u)
        else:
            outputs = _squash(jnp.einsum('bcr,bcrd->bcd', c, u))
            b = b + jnp.einsum('bcd,bcrd->bcr', outputs, u)
    return outputs


_pmapped = jax.pmap(_shard_compute, in_axes=(0, None), devices=jax.devices()[:N_CORES])


def kernel(x: np.ndarray, W: np.ndarray) -> np.ndarray:
    x = np.asarray(x, dtype=np.float32)
    W = np.asarray(W, dtype=np.float32)
    batch, N, ic, idim = x.shape
    # shard rows (node-contiguous) across cores
    xm = x.reshape(batch * N, ic, idim)
    xs = xm.reshape(N_CORES, ROWS_SHARD, ic, idim)
    out = _pmapped(xs, W)                      # [8, BP_SHARD, C, D]
    out = np.asarray(out)
    return out.reshape(batch, N // NUM_NEIGHBOURS, NUM_CAPSULE, DIM_CAPSULE)
